# revision 7
# baseline (speedup 1.0000x reference)
"""Multi-head attention (B=2, S=2048, D=1024, H=16) on 8 Trainium2 NeuronCores.

Sharding: core c -> (batch b = c // 4, head-group hg = c % 4, 4 heads each).
Each core computes its 4 heads' attention for its batch plus the partial
output projection (rows of w_o.T for its head dims). Host sums the 4 partial
outputs per batch and adds the bias constants.

v2 (this file) vs baseline:
  - all matmul operands bf16 (PE streams 1 col/cycle regardless of dtype,
    but bf16 halves DMA + SBUF traffic and enables FWL weight loads)
  - attention inner loop software-pipelined with lag L=3: the attnV matmul
    for kt trails the scores matmul for kt+3, so the in-order PE queue never
    head-blocks waiting on the Act-engine exp -> keeps HAM warm (2.4 GHz)
  - reciprocal_approx_fast (single custom-DVE op) instead of the iterative
    [1,512] reciprocal (3.3us -> ~0.7us)
  - out-projection of q-tile qt interleaved into the next block's step loop
    on its own PSUM bank pair, PSUM budget: 2x sc(2 banks) + av0 + av1 +
    2x py = 8 banks
  - ScalarE runs exp only (no table-set switches); PSUM->SBUF copies on DVE
"""
import sys

sys.path.insert(0, "/opt/trn_rl_repo")

import numpy as np
import ml_dtypes

BF16 = ml_dtypes.bfloat16

N_CORES = 8
B, S, D = 2, 2048, 1024
H, DH = 16, 64
DLOC = D // 4  # 256 head dims per core
QT = 512  # q tile (moving dim)
NQT = S // QT  # 4
KT = 128  # k positions per scores tile
NKT = S // KT  # 16
KC = D // 128  # 8 contraction chunks for projections
LAG = 3  # attnV trails scores by LAG kt-steps

_CACHE = {}


def _build():
    from concourse import bacc
    import concourse.mybir as mybir
    import concourse.tile as tile
    import concourse.bass as bass_mod

    f32 = mybir.dt.float32
    bf16 = mybir.dt.bfloat16
    AF = mybir.ActivationFunctionType

    nc = bacc.Bacc("TRN2", target_bir_lowering=False, debug=False,
                   num_devices=N_CORES)
    xq = nc.declare_dram_parameter("xq", [NQT, 128, KC, QT], bf16,
                                   isOutput=False)
    xk = nc.declare_dram_parameter("xk", [NQT, 128, KC, QT], bf16,
                                   isOutput=False)
    xv = nc.declare_dram_parameter("xv", [NQT, 128, KC, QT], bf16,
                                   isOutput=False)
    wq = nc.declare_dram_parameter("wq", [128, KC, DLOC], bf16, isOutput=False)
    wk = nc.declare_dram_parameter("wk", [128, KC, DLOC], bf16, isOutput=False)
    wv = nc.declare_dram_parameter("wv", [128, KC, DLOC], bf16, isOutput=False)
    wo = nc.declare_dram_parameter("wo", [128, 2, D], bf16, isOutput=False)
    bq = nc.declare_dram_parameter("bq", [128, 2], f32, isOutput=False)
    bk = nc.declare_dram_parameter("bk", [128, 2], f32, isOutput=False)
    yT = nc.declare_dram_parameter("yT", [8, 128, S], bf16, isOutput=True)

    with tile.TileContext(nc) as tc:
        with (
            tc.tile_pool(name="keep", bufs=1) as keep,
            tc.tile_pool(name="big", bufs=1) as big,
            tc.tile_pool(name="small", bufs=2) as small,
            tc.tile_pool(name="ypool", bufs=2) as ypool,
        ):
            # --- resident weights / constants ---
            wo_t = keep.tile([128, 2, D], bf16)
            nc.sync.dma_start(out=wo_t, in_=wo[:, :, :])
            bq_t = keep.tile([128, 2], f32)
            bk_t = keep.tile([128, 2], f32)
            nc.sync.dma_start(out=bq_t, in_=bq[:, :])
            nc.sync.dma_start(out=bk_t, in_=bk[:, :])

            # projections output (resident through attention)
            qh = big.tile([128, 2, S], bf16)   # [part, hp, q]
            kh = big.tile([128, 2, S], bf16)
            vsb = big.tile([128, NKT, 4, 65], bf16)  # [kpart, kt, head, d|1]
            aoT = big.tile([128, 2, S], bf16)  # attn out^T [dlocal, q]
            ones_sb = keep.tile([128, 4], bf16)
            nc.vector.memset(ones_sb, 1.0)
            for kt in range(NKT):
                nc.vector.tensor_copy(vsb[:, kt, :, 64], ones_sb)

            # --- phase 1: projections ---
            with (
                tc.tile_pool(name="wpool", bufs=1) as wpool,
                tc.tile_pool(name="xpool", bufs=3) as xpool,
                tc.tile_pool(name="pp", bufs=2, space="PSUM") as pp,
            ):
                wq_t = wpool.tile([128, KC, DLOC], bf16, tag="wq")
                wk_t = wpool.tile([128, KC, DLOC], bf16, tag="wk")
                wv_t = wpool.tile([128, KC, DLOC], bf16, tag="wv")
                for w_t, w_d in ((wq_t, wq), (wk_t, wk), (wv_t, wv)):
                    nc.sync.dma_start(out=w_t, in_=w_d[:, :, :])

                for x_d, x_q, w_t, o_t, b_t in (
                    (xq, nc.gpsimd, wq_t, qh, bq_t),
                    (xk, nc.sync, wk_t, kh, bk_t),
                ):
                    for nt in range(NQT):
                        xc = xpool.tile([128, KC, QT], bf16, tag="xc")
                        x_q.dma_start(
                            out=xc[:, 0:KC // 2, :],
                            in_=x_d[nt, :, 0:KC // 2, :])
                        x_q.dma_start(
                            out=xc[:, KC // 2:KC, :],
                            in_=x_d[nt, :, KC // 2:KC, :])
                        for mt in range(2):
                            ps = pp.tile([128, QT], f32, tag="p")
                            for c in range(KC):
                                nc.tensor.matmul(
                                    ps,
                                    w_t[:, c, mt * 128:(mt + 1) * 128],
                                    xc[:, c, :],
                                    start=(c == 0), stop=(c == KC - 1))
                            nc.vector.tensor_scalar_add(
                                o_t[:, mt, nt * QT:(nt + 1) * QT],
                                ps, b_t[:, mt:mt + 1])

                # V projection, natural layout, per-head slices into vsb
                for nt in range(NQT):
                    xc = xpool.tile([128, KC, QT], bf16, tag="xc")
                    nc.scalar.dma_start(
                        out=xc[:, 0:KC // 2, :],
                        in_=xv[nt, :, 0:KC // 2, :])
                    nc.scalar.dma_start(
                        out=xc[:, KC // 2:KC, :],
                        in_=xv[nt, :, KC // 2:KC, :])
                    for stl in range(QT // 128):
                        st = nt * (QT // 128) + stl
                        psv = pp.tile([128, DLOC], f32, tag="pv")
                        for c in range(KC):
                            nc.tensor.matmul(
                                psv,
                                xc[:, c, stl * 128:(stl + 1) * 128],
                                wv_t[:, c, :],
                                start=(c == 0), stop=(c == KC - 1))
                        nc.vector.tensor_copy(
                            vsb[:, st, :, 0:64],
                            psv[:].rearrange("p (h d) -> p h d", h=4))

            # --- phase 2: attention, software-pipelined ---
            with (
                tc.tile_pool(name="att", bufs=2) as att,
                tc.tile_pool(name="psS", bufs=3, space="PSUM") as psS,
                tc.tile_pool(name="psA", bufs=2, space="PSUM") as psA,
                tc.tile_pool(name="psY", bufs=1, space="PSUM") as psY,
            ):
                def outproj(qt, mt):
                    py = psY.tile([128, QT], f32, tag="py")
                    for c in range(2):
                        nc.tensor.matmul(
                            py,
                            wo_t[:, c, mt * 128:(mt + 1) * 128],
                            aoT[:, c, qt * QT:(qt + 1) * QT],
                            start=(c == 0), stop=(c == 1))
                    ysb = ypool.tile([128, QT], bf16, tag="y")
                    nc.vector.tensor_copy(ysb, py)
                    nc.sync.dma_start(
                        out=yT[mt, :, qt * QT:(qt + 1) * QT], in_=ysb)

                for qt in range(NQT):
                    for hp in range(2):  # head pairs (2h, 2h+1)
                        esb = att.tile([128, NKT, 2, QT], bf16, tag="esb")
                        av0 = psA.tile([128, QT], f32, tag="av0")
                        av1 = psA.tile([128, QT], f32, tag="av1")
                        for i in range(NKT + LAG):
                            if i < NKT:
                                kt = i
                                for side in (0, 1):
                                    scx = psS.tile([128, QT], f32, tag="sc")
                                    nc.tensor.matmul(
                                        scx,
                                        kh[64 * side:64 * (side + 1), hp,
                                           kt * 128:(kt + 1) * 128],
                                        qh[64 * side:64 * (side + 1), hp,
                                           qt * QT:(qt + 1) * QT],
                                        start=True, stop=True)
                                    nc.scalar.activation(
                                        esb[:, kt, side, :], scx,
                                        AF.Exp, scale=0.125)
                            if i >= LAG:
                                kt2 = i - LAG
                                nc.tensor.matmul(
                                    av0[0:65, :], vsb[:, kt2, 2 * hp, :],
                                    esb[:, kt2, 0, :],
                                    start=(kt2 == 0), stop=(kt2 == NKT - 1),
                                    skip_group_check=True)
                                nc.tensor.matmul(
                                    av1[0:65, :], vsb[:, kt2, 2 * hp + 1, :],
                                    esb[:, kt2, 1, :],
                                    start=(kt2 == 0), stop=(kt2 == NKT - 1),
                                    skip_group_check=True)
                            # out-projection of the previous q tile rides in
                            # the hp=0 block on its own PSUM banks
                            if hp == 0 and qt >= 1 and 4 <= i < 12:
                                outproj(qt - 1, i - 4)
                        for side, av in ((0, av0), (1, av1)):
                            # copy the denominator row to partition 0 first:
                            # the custom-DVE reciprocal mislowers a
                            # partition-offset input (HW-verified)
                            den = small.tile([1, QT], f32, tag="den")
                            nc.vector.tensor_copy(den, av[64:65, :])
                            rcp = small.tile([1, QT], f32, tag="rcp")
                            nc.vector.reciprocal_approx_fast(rcp, den)
                            bca = small.tile([64, QT], f32, tag="bca")
                            _rc = rcp[:]
                            nc.sync.dma_start(
                                out=bca,
                                in_=bass_mod.AP(
                                    tensor=_rc.tensor, offset=_rc.offset,
                                    ap=[[1, 1], [0, 64], [1, QT]]))
                            if side == 0:
                                nc.vector.tensor_mul(
                                    aoT[0:64, hp, qt * QT:(qt + 1) * QT],
                                    av[0:64, :], bca)
                            else:
                                scr = small.tile([64, QT], bf16, tag="scr")
                                nc.vector.tensor_mul(scr, av[0:64, :], bca)
                                nc.sync.dma_start(
                                    out=aoT[64:128, hp,
                                            qt * QT:(qt + 1) * QT],
                                    in_=scr)

                # tail: out-projection of the last q tile
                for mt in range(8):
                    outproj(NQT - 1, mt)
    nc.compile()
    return nc


def _get_nc():
    if "nc" not in _CACHE:
        _CACHE["nc"] = _build()
    return _CACHE["nc"]


def kernel(q, k, v, w_q, b_q, w_k, b_k, w_v, b_v, w_o, b_o, _trace=False):
    from concourse.bass_utils import run_bass_kernel_spmd

    q = np.asarray(q, np.float32)
    k = np.asarray(k, np.float32)
    v = np.asarray(v, np.float32)
    w_q = np.asarray(w_q, np.float32)
    w_k = np.asarray(w_k, np.float32)
    w_v = np.asarray(w_v, np.float32)
    w_o = np.asarray(w_o, np.float32)
    b_q = np.asarray(b_q, np.float32)
    b_k = np.asarray(b_k, np.float32)
    b_v = np.asarray(b_v, np.float32)
    b_o = np.asarray(b_o, np.float32)

    nc = _get_nc()

    def tile_x(x):
        # [S, D] -> [NQT, 128, KC, QT]: A[nt, p, c, s] = x[nt*QT+s, c*128+p]
        t = x.T.reshape(KC, 128, NQT, QT)
        return np.ascontiguousarray(
            t.transpose(2, 1, 0, 3)).astype(BF16)

    def tile_w(w, lo, hi):
        # [D, dloc] -> [128, KC, dloc]
        t = w[lo:hi, :].T.reshape(KC, 128, DLOC)
        return np.ascontiguousarray(t.transpose(1, 0, 2)).astype(BF16)

    xqT = [tile_x(q[b]) for b in range(B)]
    xkT = [tile_x(k[b]) for b in range(B)]
    xvT = [tile_x(v[b]) for b in range(B)]

    in_maps = []
    for c in range(N_CORES):
        b, hg = c // 4, c % 4
        lo, hi = hg * DLOC, (hg + 1) * DLOC
        in_maps.append({
            "xq": xqT[b],
            "xk": xkT[b],
            "xv": xvT[b],
            "wq": tile_w(w_q, lo, hi),
            "wk": tile_w(w_k, lo, hi),
            "wv": tile_w(w_v, lo, hi),
            "wo": np.ascontiguousarray(
                w_o[:, lo:hi].T.reshape(2, 128, D).transpose(1, 0, 2)
            ).astype(BF16),
            "bq": np.ascontiguousarray(b_q[lo:hi].reshape(2, 128).T),
            "bk": np.ascontiguousarray(b_k[lo:hi].reshape(2, 128).T),
        })

    res = run_bass_kernel_spmd(
        nc, in_maps, core_ids=list(range(N_CORES)), trace=_trace)
    if _trace:
        _CACHE["last_result"] = res

    # b_v contributes exactly (w_o @ b_v) per output element (softmax rows
    # sum to 1); b_o adds directly.
    const_row = (b_o + w_o @ b_v).astype(np.float32)  # [D]
    out = np.empty((B, S, D), np.float32)
    for b in range(B):
        acc = res.results[4 * b]["yT"].astype(np.float32)
        for c in range(4 * b + 1, 4 * b + 4):
            acc += res.results[c]["yT"].astype(np.float32)
        out[b] = acc.reshape(D, S).T + const_row
    return out


# revision 11
# speedup vs baseline: 1.0802x; 1.0802x over previous
"""Multi-head attention (B=2, S=2048, D=1024, H=16) on 8 Trainium2 NeuronCores.

Sharding: core c -> (batch b = c // 4, head-group hg = c % 4, 4 heads each).
Each core computes its 4 heads' attention for its batch plus the partial
output projection (rows of w_o.T for its head dims). Host sums the 4 partial
outputs per batch and adds the bias constants.

v2 (this file) vs baseline:
  - all matmul operands bf16 (PE streams 1 col/cycle regardless of dtype,
    but bf16 halves DMA + SBUF traffic and enables FWL weight loads)
  - attention inner loop software-pipelined with lag L=3: the attnV matmul
    for kt trails the scores matmul for kt+3, so the in-order PE queue never
    head-blocks waiting on the Act-engine exp -> keeps HAM warm (2.4 GHz)
  - reciprocal_approx_fast (single custom-DVE op) instead of the iterative
    [1,512] reciprocal (3.3us -> ~0.7us)
  - out-projection of q-tile qt interleaved into the next block's step loop
    on its own PSUM bank pair, PSUM budget: 2x sc(2 banks) + av0 + av1 +
    2x py = 8 banks
  - ScalarE runs exp only (no table-set switches); PSUM->SBUF copies on DVE
"""
import sys

sys.path.insert(0, "/opt/trn_rl_repo")

import numpy as np
import ml_dtypes

BF16 = ml_dtypes.bfloat16

N_CORES = 8
B, S, D = 2, 2048, 1024
H, DH = 16, 64
DLOC = D // 4  # 256 head dims per core
QT = 512  # q tile (moving dim)
NQT = S // QT  # 4
KT = 128  # k positions per scores tile
NKT = S // KT  # 16
KC = D // 128  # 8 contraction chunks for projections
LAG = 3  # attnV trails scores by LAG kt-steps

_CACHE = {}


def _build():
    from concourse import bacc
    import concourse.mybir as mybir
    import concourse.tile as tile
    import concourse.bass as bass_mod

    f32 = mybir.dt.float32
    bf16 = mybir.dt.bfloat16
    AF = mybir.ActivationFunctionType

    nc = bacc.Bacc("TRN2", target_bir_lowering=False, debug=False,
                   num_devices=N_CORES)
    xq = nc.declare_dram_parameter("xq", [NQT, 128, KC, QT], bf16,
                                   isOutput=False)
    xk = nc.declare_dram_parameter("xk", [NQT, 128, KC, QT], bf16,
                                   isOutput=False)
    xv = nc.declare_dram_parameter("xv", [NQT, 128, KC, QT], bf16,
                                   isOutput=False)
    wq = nc.declare_dram_parameter("wq", [128, KC, DLOC], bf16, isOutput=False)
    wk = nc.declare_dram_parameter("wk", [128, KC, DLOC], bf16, isOutput=False)
    wv = nc.declare_dram_parameter("wv", [128, KC, DLOC], bf16, isOutput=False)
    wo = nc.declare_dram_parameter("wo", [128, 2, D], bf16, isOutput=False)
    bq = nc.declare_dram_parameter("bq", [128, 2], f32, isOutput=False)
    bk = nc.declare_dram_parameter("bk", [128, 2], f32, isOutput=False)
    yT = nc.declare_dram_parameter("yT", [8, 128, S], bf16, isOutput=True)

    with tile.TileContext(nc) as tc:
        with (
            tc.tile_pool(name="keep", bufs=1) as keep,
            tc.tile_pool(name="big", bufs=1) as big,
            tc.tile_pool(name="small", bufs=2) as small,
            tc.tile_pool(name="ypool", bufs=2) as ypool,
        ):
            # --- resident weights / constants ---
            wo_t = keep.tile([128, 2, D], bf16)
            nc.sync.dma_start(out=wo_t, in_=wo[:, :, :])
            bq_t = keep.tile([128, 2], f32)
            bk_t = keep.tile([128, 2], f32)
            nc.sync.dma_start(out=bq_t, in_=bq[:, :])
            nc.sync.dma_start(out=bk_t, in_=bk[:, :])

            # projections output (resident through attention)
            qh = big.tile([128, 2, S], bf16)   # [part, hp, q]
            kh = big.tile([128, 2, S], bf16)
            vsb = big.tile([128, NKT, 4, 65], bf16)  # [kpart, kt, head, d|1]
            aoT = big.tile([128, 2, S], bf16)  # attn out^T [dlocal, q]
            ones_sb = keep.tile([128, 4], bf16)
            nc.vector.memset(ones_sb, 1.0)
            for kt in range(NKT):
                nc.vector.tensor_copy(vsb[:, kt, :, 64], ones_sb)

            # --- phase 1: projections ---
            with (
                tc.tile_pool(name="wpool", bufs=1) as wpool,
                tc.tile_pool(name="xpool", bufs=3) as xpool,
                tc.tile_pool(name="pp", bufs=2, space="PSUM") as pp,
            ):
                wq_t = wpool.tile([128, KC, DLOC], bf16, tag="wq")
                wk_t = wpool.tile([128, KC, DLOC], bf16, tag="wk")
                wv_t = wpool.tile([128, KC, DLOC], bf16, tag="wv")
                for w_t, w_d in ((wq_t, wq), (wk_t, wk), (wv_t, wv)):
                    nc.sync.dma_start(out=w_t, in_=w_d[:, :, :])

                for x_d, x_q, w_t, o_t, b_t in (
                    (xq, nc.gpsimd, wq_t, qh, bq_t),
                    (xk, nc.sync, wk_t, kh, bk_t),
                ):
                    for nt in range(NQT):
                        xc = xpool.tile([128, KC, QT], bf16, tag="xc")
                        x_q.dma_start(
                            out=xc[:, 0:KC // 2, :],
                            in_=x_d[nt, :, 0:KC // 2, :])
                        x_q.dma_start(
                            out=xc[:, KC // 2:KC, :],
                            in_=x_d[nt, :, KC // 2:KC, :])
                        for mt in range(2):
                            ps = pp.tile([128, QT], f32, tag="p")
                            for c in range(KC):
                                nc.tensor.matmul(
                                    ps,
                                    w_t[:, c, mt * 128:(mt + 1) * 128],
                                    xc[:, c, :],
                                    start=(c == 0), stop=(c == KC - 1))
                            nc.vector.tensor_scalar_add(
                                o_t[:, mt, nt * QT:(nt + 1) * QT],
                                ps, b_t[:, mt:mt + 1])

                # V projection, natural layout, per-head slices into vsb
                for nt in range(NQT):
                    xc = xpool.tile([128, KC, QT], bf16, tag="xc")
                    nc.scalar.dma_start(
                        out=xc[:, 0:KC // 2, :],
                        in_=xv[nt, :, 0:KC // 2, :])
                    nc.scalar.dma_start(
                        out=xc[:, KC // 2:KC, :],
                        in_=xv[nt, :, KC // 2:KC, :])
                    for stl in range(QT // 128):
                        st = nt * (QT // 128) + stl
                        psv = pp.tile([128, DLOC], f32, tag="pv")
                        for c in range(KC):
                            nc.tensor.matmul(
                                psv,
                                xc[:, c, stl * 128:(stl + 1) * 128],
                                wv_t[:, c, :],
                                start=(c == 0), stop=(c == KC - 1))
                        nc.vector.tensor_copy(
                            vsb[:, st, :, 0:64],
                            psv[:].rearrange("p (h d) -> p h d", h=4))

            # --- phase 2: attention, software-pipelined ---
            with (
                tc.tile_pool(name="att", bufs=2) as att,
                tc.tile_pool(name="avp", bufs=2) as avp,
                tc.tile_pool(name="psS", bufs=2, space="PSUM") as psS,
                tc.tile_pool(name="psA", bufs=1, space="PSUM") as psA,
                tc.tile_pool(name="psY", bufs=2, space="PSUM") as psY,
            ):
                def outproj(qt, mt):
                    py = psY.tile([128, QT], f32, tag="py")
                    for c in range(2):
                        nc.tensor.matmul(
                            py,
                            wo_t[:, c, mt * 128:(mt + 1) * 128],
                            aoT[:, c, qt * QT:(qt + 1) * QT],
                            start=(c == 0), stop=(c == 1))
                    ysb = ypool.tile([128, QT], bf16, tag="y")
                    nc.vector.tensor_copy(ysb, py)
                    # gpsimd DMA queue: keeps sync free for the bca broadcast
                    nc.gpsimd.dma_start(
                        out=yT[mt, :, qt * QT:(qt + 1) * QT], in_=ysb)

                for qt in range(NQT):
                    for hp in range(2):  # head pairs (2h, 2h+1)
                        esb = att.tile([128, NKT, 2, QT], bf16, tag="esb")
                        av0 = psA.tile([128, QT], f32, tag="av0")
                        av1 = psA.tile([128, QT], f32, tag="av1")
                        for i in range(NKT + LAG):
                            if i < NKT:
                                kt = i
                                sc = psS.tile([128, 2 * QT], f32, tag="sc")
                                nc.tensor.matmul(
                                    sc[:, 0:QT],
                                    kh[0:64, hp, kt * 128:(kt + 1) * 128],
                                    qh[0:64, hp, qt * QT:(qt + 1) * QT],
                                    start=True, stop=True)
                                nc.tensor.matmul(
                                    sc[:, QT:2 * QT],
                                    kh[64:128, hp, kt * 128:(kt + 1) * 128],
                                    qh[64:128, hp, qt * QT:(qt + 1) * QT],
                                    start=True, stop=True)
                                nc.scalar.activation(
                                    esb[:, kt, :, :], sc, AF.Exp, scale=0.125)
                            if i >= LAG:
                                kt2 = i - LAG
                                nc.tensor.matmul(
                                    av0[0:65, :], vsb[:, kt2, 2 * hp, :],
                                    esb[:, kt2, 0, :],
                                    start=(kt2 == 0), stop=(kt2 == NKT - 1),
                                    skip_group_check=True)
                                nc.tensor.matmul(
                                    av1[0:65, :], vsb[:, kt2, 2 * hp + 1, :],
                                    esb[:, kt2, 1, :],
                                    start=(kt2 == 0), stop=(kt2 == NKT - 1),
                                    skip_group_check=True)
                            # out-projection of the previous q tile rides in
                            # the hp=0 block on its own PSUM banks
                            if hp == 0 and qt >= 1 and 4 <= i < 12:
                                outproj(qt - 1, i - 4)
                        for side, av in ((0, av0), (1, av1)):
                            # evacuate av from PSUM in ONE op so the bank is
                            # reusable by the next block immediately; the
                            # normalize chain then runs all-SBUF off the
                            # critical path
                            avs = avp.tile([128, QT], f32, tag="avs")
                            nc.vector.tensor_copy(avs, av)
                            # denominator to partition 0 (custom-DVE recip
                            # mislowers a partition-offset input, HW-verified)
                            den = small.tile([1, QT], f32, tag="den")
                            nc.vector.tensor_copy(den, avs[64:65, :])
                            rcp = small.tile([1, QT], f32, tag="rcp")
                            nc.vector.reciprocal_approx_fast(rcp, den)
                            bca = small.tile([64, QT], f32, tag="bca")
                            _rc = rcp[:]
                            nc.sync.dma_start(
                                out=bca,
                                in_=bass_mod.AP(
                                    tensor=_rc.tensor, offset=_rc.offset,
                                    ap=[[1, 1], [0, 64], [1, QT]]))
                            if side == 0:
                                nc.vector.tensor_mul(
                                    aoT[0:64, hp, qt * QT:(qt + 1) * QT],
                                    avs[0:64, :], bca)
                            else:
                                scr = small.tile([64, QT], bf16, tag="scr")
                                nc.vector.tensor_mul(scr, avs[0:64, :], bca)
                                nc.sync.dma_start(
                                    out=aoT[64:128, hp,
                                            qt * QT:(qt + 1) * QT],
                                    in_=scr)

                # tail: out-projection of the last q tile
                for mt in range(8):
                    outproj(NQT - 1, mt)
    nc.compile()
    return nc


def _get_nc():
    if "nc" not in _CACHE:
        _CACHE["nc"] = _build()
    return _CACHE["nc"]


def kernel(q, k, v, w_q, b_q, w_k, b_k, w_v, b_v, w_o, b_o, _trace=False):
    from concourse.bass_utils import run_bass_kernel_spmd

    q = np.asarray(q, np.float32)
    k = np.asarray(k, np.float32)
    v = np.asarray(v, np.float32)
    w_q = np.asarray(w_q, np.float32)
    w_k = np.asarray(w_k, np.float32)
    w_v = np.asarray(w_v, np.float32)
    w_o = np.asarray(w_o, np.float32)
    b_q = np.asarray(b_q, np.float32)
    b_k = np.asarray(b_k, np.float32)
    b_v = np.asarray(b_v, np.float32)
    b_o = np.asarray(b_o, np.float32)

    nc = _get_nc()

    def tile_x(x):
        # [S, D] -> [NQT, 128, KC, QT]: A[nt, p, c, s] = x[nt*QT+s, c*128+p]
        t = x.T.reshape(KC, 128, NQT, QT)
        return np.ascontiguousarray(
            t.transpose(2, 1, 0, 3)).astype(BF16)

    def tile_w(w, lo, hi):
        # [D, dloc] -> [128, KC, dloc]
        t = w[lo:hi, :].T.reshape(KC, 128, DLOC)
        return np.ascontiguousarray(t.transpose(1, 0, 2)).astype(BF16)

    xqT = [tile_x(q[b]) for b in range(B)]
    xkT = [tile_x(k[b]) for b in range(B)]
    xvT = [tile_x(v[b]) for b in range(B)]

    in_maps = []
    for c in range(N_CORES):
        b, hg = c // 4, c % 4
        lo, hi = hg * DLOC, (hg + 1) * DLOC
        in_maps.append({
            "xq": xqT[b],
            "xk": xkT[b],
            "xv": xvT[b],
            "wq": tile_w(w_q, lo, hi),
            "wk": tile_w(w_k, lo, hi),
            "wv": tile_w(w_v, lo, hi),
            "wo": np.ascontiguousarray(
                w_o[:, lo:hi].T.reshape(2, 128, D).transpose(1, 0, 2)
            ).astype(BF16),
            "bq": np.ascontiguousarray(b_q[lo:hi].reshape(2, 128).T),
            "bk": np.ascontiguousarray(b_k[lo:hi].reshape(2, 128).T),
        })

    res = run_bass_kernel_spmd(
        nc, in_maps, core_ids=list(range(N_CORES)), trace=_trace)
    if _trace:
        _CACHE["last_result"] = res

    # b_v contributes exactly (w_o @ b_v) per output element (softmax rows
    # sum to 1); b_o adds directly.
    const_row = (b_o + w_o @ b_v).astype(np.float32)  # [D]
    out = np.empty((B, S, D), np.float32)
    for b in range(B):
        acc = res.results[4 * b]["yT"].astype(np.float32)
        for c in range(4 * b + 1, 4 * b + 4):
            acc += res.results[c]["yT"].astype(np.float32)
        out[b] = acc.reshape(D, S).T + const_row
    return out


# revision 22
# speedup vs baseline: 1.3674x; 1.2659x over previous
"""Multi-head attention (B=2, S=2048, D=1024, H=16) on 8 Trainium2 NeuronCores.

Sharding: core c -> (batch b = c // 4, head-group hg = c % 4, 4 heads each).
Each core computes its 4 heads' attention for its batch plus the partial
output projection (rows of w_o.T for its head dims). Host sums the 4 partial
outputs per batch and adds the bias constants.

v2 (this file) vs baseline:
  - all matmul operands bf16 (PE streams 1 col/cycle regardless of dtype,
    but bf16 halves DMA + SBUF traffic and enables FWL weight loads)
  - attention inner loop software-pipelined with lag L=3: the attnV matmul
    for kt trails the scores matmul for kt+3, so the in-order PE queue never
    head-blocks waiting on the Act-engine exp -> keeps HAM warm (2.4 GHz)
  - reciprocal_approx_fast (single custom-DVE op) instead of the iterative
    [1,512] reciprocal (3.3us -> ~0.7us)
  - out-projection of q-tile qt interleaved into the next block's step loop
    on its own PSUM bank pair, PSUM budget: 2x sc(2 banks) + av0 + av1 +
    2x py = 8 banks
  - ScalarE runs exp only (no table-set switches); PSUM->SBUF copies on DVE
"""
import sys

sys.path.insert(0, "/opt/trn_rl_repo")

import numpy as np
import ml_dtypes

BF16 = ml_dtypes.bfloat16

N_CORES = 8
B, S, D = 2, 2048, 1024
H, DH = 16, 64
DLOC = D // 4  # 256 head dims per core
QT = 512  # q tile (moving dim)
NQT = S // QT  # 4
KT = 128  # k positions per scores tile
NKT = S // KT  # 16
KC = D // 128  # 8 contraction chunks for projections
LAG = 3  # attnV trails scores by LAG kt-steps

_CACHE = {}


def _build():
    from concourse import bacc
    import concourse.mybir as mybir
    import concourse.tile as tile
    import concourse.bass as bass_mod

    f32 = mybir.dt.float32
    bf16 = mybir.dt.bfloat16
    AF = mybir.ActivationFunctionType

    nc = bacc.Bacc("TRN2", target_bir_lowering=False, debug=False,
                   num_devices=N_CORES)
    xq = nc.declare_dram_parameter("xq", [NQT, 128, KC, QT], bf16,
                                   isOutput=False)
    xk = nc.declare_dram_parameter("xk", [NQT, 128, KC, QT], bf16,
                                   isOutput=False)
    xv = nc.declare_dram_parameter("xv", [NQT, 128, KC, QT], bf16,
                                   isOutput=False)
    wq = nc.declare_dram_parameter("wq", [128, KC, DLOC], bf16, isOutput=False)
    wk = nc.declare_dram_parameter("wk", [128, KC, DLOC], bf16, isOutput=False)
    wv = nc.declare_dram_parameter("wv", [128, KC, DLOC], bf16, isOutput=False)
    wo = nc.declare_dram_parameter("wo", [128, 2, D], bf16, isOutput=False)
    bq = nc.declare_dram_parameter("bq", [128, 2], f32, isOutput=False)
    bk = nc.declare_dram_parameter("bk", [128, 2], f32, isOutput=False)
    # [mt, qt, 128, QT] so each store is one contiguous 128KB chunk
    yT = nc.declare_dram_parameter("yT", [8, NQT, 128, QT], bf16,
                                   isOutput=True)

    with tile.TileContext(nc) as tc:
        with (
            tc.tile_pool(name="keep", bufs=1) as keep,
            tc.tile_pool(name="big", bufs=1) as big,
            tc.tile_pool(name="small", bufs=2) as small,
            tc.tile_pool(name="ypool", bufs=2) as ypool,
        ):
            # --- resident weights / constants ---
            wo_t = keep.tile([128, 2, D], bf16)
            nc.sync.dma_start(out=wo_t, in_=wo[:, :, :])
            bq_t = keep.tile([128, 2], f32)
            bk_t = keep.tile([128, 2], f32)
            nc.sync.dma_start(out=bq_t, in_=bq[:, :])
            nc.sync.dma_start(out=bk_t, in_=bk[:, :])

            # projections output (resident through attention)
            qh = big.tile([128, 2, S], bf16)   # [part, hp, q]
            kh = big.tile([128, 2, S], bf16)
            vsb = big.tile([128, NKT, 4, 65], bf16)  # [kpart, kt, head, d|1]
            aoT = big.tile([128, 2, S], bf16)  # attn out^T [dlocal, q]
            ones_sb = keep.tile([128, 4], bf16)
            nc.vector.memset(ones_sb, 1.0)
            for kt in range(NKT):
                nc.vector.tensor_copy(vsb[:, kt, :, 64], ones_sb)
            f32r = mybir.dt.float32r
            # lhsT of the rcp broadcast MM (f32r memset mis-compiles, so
            # memset f32 and copy)
            ones_f = keep.tile([1, 64], f32)
            nc.vector.memset(ones_f, 1.0)
            ones_r = keep.tile([1, 64], f32r)
            nc.vector.tensor_copy(ones_r, ones_f)

            # --- phase 1: projections ---
            with (
                tc.tile_pool(name="wpool", bufs=1) as wpool,
                tc.tile_pool(name="xpool", bufs=3) as xpool,
                tc.tile_pool(name="pp", bufs=2, space="PSUM") as pp,
            ):
                wq_t = wpool.tile([128, KC, DLOC], bf16, tag="wq")
                wk_t = wpool.tile([128, KC, DLOC], bf16, tag="wk")
                wv_t = wpool.tile([128, KC, DLOC], bf16, tag="wv")
                for w_t, w_d in ((wq_t, wq), (wk_t, wk), (wv_t, wv)):
                    nc.sync.dma_start(out=w_t, in_=w_d[:, :, :])

                for x_d, x_q, w_t, o_t, b_t in (
                    (xq, nc.gpsimd, wq_t, qh, bq_t),
                    (xk, nc.sync, wk_t, kh, bk_t),
                ):
                    for nt in range(NQT):
                        xc = xpool.tile([128, KC, QT], bf16, tag="xc")
                        x_q.dma_start(
                            out=xc[:, 0:KC // 2, :],
                            in_=x_d[nt, :, 0:KC // 2, :])
                        x_q.dma_start(
                            out=xc[:, KC // 2:KC, :],
                            in_=x_d[nt, :, KC // 2:KC, :])
                        for mt in range(2):
                            ps = pp.tile([128, QT], f32, tag="p")
                            for c in range(KC):
                                nc.tensor.matmul(
                                    ps,
                                    w_t[:, c, mt * 128:(mt + 1) * 128],
                                    xc[:, c, :],
                                    start=(c == 0), stop=(c == KC - 1))
                            nc.vector.tensor_scalar_add(
                                o_t[:, mt, nt * QT:(nt + 1) * QT],
                                ps, b_t[:, mt:mt + 1])

                # V projection, natural layout, per-head slices into vsb
                for nt in range(NQT):
                    xc = xpool.tile([128, KC, QT], bf16, tag="xc")
                    nc.scalar.dma_start(
                        out=xc[:, 0:KC // 2, :],
                        in_=xv[nt, :, 0:KC // 2, :])
                    nc.scalar.dma_start(
                        out=xc[:, KC // 2:KC, :],
                        in_=xv[nt, :, KC // 2:KC, :])
                    for stl in range(QT // 128):
                        st = nt * (QT // 128) + stl
                        psv = pp.tile([128, DLOC], f32, tag="pv")
                        for c in range(KC):
                            nc.tensor.matmul(
                                psv,
                                xc[:, c, stl * 128:(stl + 1) * 128],
                                wv_t[:, c, :],
                                start=(c == 0), stop=(c == KC - 1))
                        nc.vector.tensor_copy(
                            vsb[:, st, :, 0:64],
                            psv[:].rearrange("p (h d) -> p h d", h=4))

            # --- phase 2: attention, software-pipelined ---
            with (
                tc.tile_pool(name="att", bufs=2) as att,
                tc.tile_pool(name="avp", bufs=2) as avp,
                tc.tile_pool(name="psS", bufs=2, space="PSUM") as psS,
                tc.tile_pool(name="psA", bufs=1, space="PSUM") as psA,
                tc.tile_pool(name="psY", bufs=2, space="PSUM") as psY,
            ):
                def outproj(qt, mt):
                    py = psY.tile([128, QT], f32, tag="py")
                    for c in range(2):
                        nc.tensor.matmul(
                            py,
                            wo_t[:, c, mt * 128:(mt + 1) * 128],
                            aoT[:, c, qt * QT:(qt + 1) * QT],
                            start=(c == 0), stop=(c == 1))
                    ysb = ypool.tile([128, QT], bf16, tag="y")
                    nc.vector.tensor_copy(ysb, py)
                    # gpsimd DMA queue: keeps sync free for the bca broadcast
                    nc.gpsimd.dma_start(out=yT[mt, qt], in_=ysb)

                def normalize(qt, hp, side, avs, rcr):
                    # rcp broadcast via a K=1 matmul (replaces the slow
                    # 64-descriptor broadcast DMA): bps[m, q] = rcp[q]
                    bps = psY.tile([128, QT], f32, tag="py")
                    nc.tensor.matmul(bps[0:64, :], ones_r, rcr,
                                     start=True, stop=True)
                    if side == 0:
                        nc.vector.tensor_mul(
                            aoT[0:64, hp, qt * QT:(qt + 1) * QT],
                            avs[0:64, :], bps[0:64, :])
                    else:
                        scr = small.tile([64, QT], bf16, tag="scr")
                        nc.vector.tensor_mul(scr, avs[0:64, :], bps[0:64, :])
                        nc.sync.dma_start(
                            out=aoT[64:128, hp, qt * QT:(qt + 1) * QT],
                            in_=scr)

                # epilogue work deferred into the NEXT block's step loop so
                # the in-order PE queue never waits on the DVE chain
                pending = []

                for qt in range(NQT):
                    for hp in range(2):  # head pairs (2h, 2h+1)
                        esb = att.tile([128, NKT, 2, QT], bf16, tag="esb")
                        av0 = psA.tile([128, QT], f32, tag="av0")
                        av1 = psA.tile([128, QT], f32, tag="av1")
                        deferred, pending = pending, []
                        for i in range(NKT + LAG):
                            if i < NKT:
                                kt = i
                                sc = psS.tile([128, 2 * QT], f32, tag="sc")
                                nc.tensor.matmul(
                                    sc[:, 0:QT],
                                    kh[0:64, hp, kt * 128:(kt + 1) * 128],
                                    qh[0:64, hp, qt * QT:(qt + 1) * QT],
                                    start=True, stop=True)
                                nc.tensor.matmul(
                                    sc[:, QT:2 * QT],
                                    kh[64:128, hp, kt * 128:(kt + 1) * 128],
                                    qh[64:128, hp, qt * QT:(qt + 1) * QT],
                                    start=True, stop=True)
                                nc.scalar.activation(
                                    esb[:, kt, :, :], sc, AF.Exp, scale=0.125)
                            if i >= LAG:
                                kt2 = i - LAG
                                nc.tensor.matmul(
                                    av0[0:65, :], vsb[:, kt2, 2 * hp, :],
                                    esb[:, kt2, 0, :],
                                    start=(kt2 == 0), stop=(kt2 == NKT - 1),
                                    skip_group_check=True)
                                nc.tensor.matmul(
                                    av1[0:65, :], vsb[:, kt2, 2 * hp + 1, :],
                                    esb[:, kt2, 1, :],
                                    start=(kt2 == 0), stop=(kt2 == NKT - 1),
                                    skip_group_check=True)
                            # previous block's normalize at steps 3/5 (its
                            # DVE chain is done by then)
                            if i == 3 and len(deferred) > 0:
                                deferred[0]()
                            if i == 5 and len(deferred) > 1:
                                deferred[1]()
                            # out-projection of the previous q tile: 4 mts
                            # in each of qt's two blocks (aoT(qt-1) is fully
                            # normalized by step 5 of the hp=0 block)
                            if qt >= 1 and 6 <= i < 10:
                                outproj(qt - 1, (i - 6) + 4 * hp)
                        for side, av in ((0, av0), (1, av1)):
                            # evacuate av from PSUM in ONE op so the bank is
                            # reusable by the next block immediately
                            avs = avp.tile([128, QT], f32, tag="avs")
                            nc.vector.tensor_copy(avs, av)
                            # denominator to partition 0 (custom-DVE recip
                            # mislowers a partition-offset input)
                            den = small.tile([1, QT], f32, tag="den")
                            nc.vector.tensor_copy(den, avs[64:65, :])
                            rcp = small.tile([1, QT], f32, tag="rcp")
                            nc.vector.reciprocal_approx_fast(rcp, den)
                            rcr = small.tile([1, QT], f32r, tag="rcr")
                            nc.vector.tensor_copy(rcr, rcp)
                            pending.append(
                                lambda q_=qt, h_=hp, s_=side, a_=avs, r_=rcr:
                                normalize(q_, h_, s_, a_, r_))

                # final flush: last block's normalize + last q tile out-proj
                for fn in pending:
                    fn()
                for mt in range(8):
                    outproj(NQT - 1, mt)
    nc.compile()
    return nc


def _get_nc():
    if "nc" not in _CACHE:
        _CACHE["nc"] = _build()
    return _CACHE["nc"]


def kernel(q, k, v, w_q, b_q, w_k, b_k, w_v, b_v, w_o, b_o, _trace=False):
    from concourse.bass_utils import run_bass_kernel_spmd

    q = np.asarray(q, np.float32)
    k = np.asarray(k, np.float32)
    v = np.asarray(v, np.float32)
    w_q = np.asarray(w_q, np.float32)
    w_k = np.asarray(w_k, np.float32)
    w_v = np.asarray(w_v, np.float32)
    w_o = np.asarray(w_o, np.float32)
    b_q = np.asarray(b_q, np.float32)
    b_k = np.asarray(b_k, np.float32)
    b_v = np.asarray(b_v, np.float32)
    b_o = np.asarray(b_o, np.float32)

    nc = _get_nc()

    def tile_x(x):
        # [S, D] -> [NQT, 128, KC, QT]: A[nt, p, c, s] = x[nt*QT+s, c*128+p]
        t = x.T.reshape(KC, 128, NQT, QT)
        return np.ascontiguousarray(
            t.transpose(2, 1, 0, 3)).astype(BF16)

    def tile_w(w, lo, hi):
        # [D, dloc] -> [128, KC, dloc]
        t = w[lo:hi, :].T.reshape(KC, 128, DLOC)
        return np.ascontiguousarray(t.transpose(1, 0, 2)).astype(BF16)

    xqT = [tile_x(q[b]) for b in range(B)]
    xkT = [tile_x(k[b]) for b in range(B)]
    xvT = [tile_x(v[b]) for b in range(B)]

    in_maps = []
    for c in range(N_CORES):
        b, hg = c // 4, c % 4
        lo, hi = hg * DLOC, (hg + 1) * DLOC
        in_maps.append({
            "xq": xqT[b],
            "xk": xkT[b],
            "xv": xvT[b],
            "wq": tile_w(w_q, lo, hi),
            "wk": tile_w(w_k, lo, hi),
            "wv": tile_w(w_v, lo, hi),
            "wo": np.ascontiguousarray(
                w_o[:, lo:hi].T.reshape(2, 128, D).transpose(1, 0, 2)
            ).astype(BF16),
            "bq": np.ascontiguousarray(b_q[lo:hi].reshape(2, 128).T),
            "bk": np.ascontiguousarray(b_k[lo:hi].reshape(2, 128).T),
        })

    res = run_bass_kernel_spmd(
        nc, in_maps, core_ids=list(range(N_CORES)), trace=_trace)
    if _trace:
        _CACHE["last_result"] = res

    # b_v contributes exactly (w_o @ b_v) per output element (softmax rows
    # sum to 1); b_o adds directly.
    const_row = (b_o + w_o @ b_v).astype(np.float32)  # [D]
    out = np.empty((B, S, D), np.float32)
    for b in range(B):
        acc = res.results[4 * b]["yT"].astype(np.float32)
        for c in range(4 * b + 1, 4 * b + 4):
            acc += res.results[c]["yT"].astype(np.float32)
        # yT[mt, qt, p, s] = y_part[mt*128+p, qt*QT+s]
        y = acc.transpose(0, 2, 1, 3).reshape(D, S)
        out[b] = y.T + const_row
    return out


# revision 28
# speedup vs baseline: 1.6132x; 1.1798x over previous
"""Multi-head attention (B=2, S=2048, D=1024, H=16) on 8 Trainium2 NeuronCores.

Sharding: core c -> (batch b = c // 4, head-group hg = c % 4, 4 heads each).
Each core computes its 4 heads' attention for its batch plus the partial
output projection (rows of w_o.T for its head dims). Host sums the 4 partial
outputs per batch and adds the bias constants.

v2 (this file) vs baseline:
  - all matmul operands bf16 (PE streams 1 col/cycle regardless of dtype,
    but bf16 halves DMA + SBUF traffic and enables FWL weight loads)
  - attention inner loop software-pipelined with lag L=3: the attnV matmul
    for kt trails the scores matmul for kt+3, so the in-order PE queue never
    head-blocks waiting on the Act-engine exp -> keeps HAM warm (2.4 GHz)
  - reciprocal_approx_fast (single custom-DVE op) instead of the iterative
    [1,512] reciprocal (3.3us -> ~0.7us)
  - out-projection of q-tile qt interleaved into the next block's step loop
    on its own PSUM bank pair, PSUM budget: 2x sc(2 banks) + av0 + av1 +
    2x py = 8 banks
  - ScalarE runs exp only (no table-set switches); PSUM->SBUF copies on DVE
"""
import sys

sys.path.insert(0, "/opt/trn_rl_repo")

import numpy as np
import ml_dtypes

BF16 = ml_dtypes.bfloat16

N_CORES = 8
B, S, D = 2, 2048, 1024
H, DH = 16, 64
DLOC = D // 4  # 256 head dims per core
QT = 512  # q tile (moving dim)
NQT = S // QT  # 4
KT = 128  # k positions per scores tile
NKT = S // KT  # 16
KC = D // 128  # 8 contraction chunks for projections
LAG = 3  # attnV trails scores by LAG kt-steps

_CACHE = {}


def _build():
    from concourse import bacc
    import concourse.mybir as mybir
    import concourse.tile as tile
    import concourse.bass as bass_mod

    f32 = mybir.dt.float32
    bf16 = mybir.dt.bfloat16
    AF = mybir.ActivationFunctionType

    nc = bacc.Bacc("TRN2", target_bir_lowering=False, debug=False,
                   num_devices=N_CORES)
    xq = nc.declare_dram_parameter("xq", [NQT, 128, KC, QT], bf16,
                                   isOutput=False)
    xk = nc.declare_dram_parameter("xk", [NQT, 128, KC, QT], bf16,
                                   isOutput=False)
    xv = nc.declare_dram_parameter("xv", [NQT, 128, KC, QT], bf16,
                                   isOutput=False)
    wq = nc.declare_dram_parameter("wq", [128, KC, DLOC], bf16, isOutput=False)
    wk = nc.declare_dram_parameter("wk", [128, KC, DLOC], bf16, isOutput=False)
    wv = nc.declare_dram_parameter("wv", [128, KC, DLOC], bf16, isOutput=False)
    wo = nc.declare_dram_parameter("wo", [128, 2, D], bf16, isOutput=False)
    bq = nc.declare_dram_parameter("bq", [128, 2], f32, isOutput=False)
    bk = nc.declare_dram_parameter("bk", [128, 2], f32, isOutput=False)
    # [mt, qt, 128, QT] so each store is one contiguous 128KB chunk
    yT = nc.declare_dram_parameter("yT", [8, NQT, 128, QT], bf16,
                                   isOutput=True)

    with tile.TileContext(nc) as tc:
        with (
            tc.tile_pool(name="keep", bufs=1) as keep,
            tc.tile_pool(name="big", bufs=1) as big,
            tc.tile_pool(name="small", bufs=2) as small,
            tc.tile_pool(name="ypool", bufs=2) as ypool,
        ):
            # --- resident weights / constants ---
            wo_t = keep.tile([128, 2, D], bf16)
            nc.sync.dma_start(out=wo_t, in_=wo[:, :, :])
            bq_t = keep.tile([128, 2], f32)
            bk_t = keep.tile([128, 2], f32)
            nc.sync.dma_start(out=bq_t, in_=bq[:, :])
            nc.sync.dma_start(out=bk_t, in_=bk[:, :])

            # projections output (resident through attention)
            qh = big.tile([128, 2, S], bf16)   # [part, hp, q]
            kh = big.tile([128, 2, S], bf16)
            vsb = big.tile([128, NKT, 4, 65], bf16)  # [kpart, kt, head, d|1]
            aoT = big.tile([128, 2, S], bf16)  # attn out^T [dlocal, q]
            ones_sb = keep.tile([128, 4], bf16)
            nc.vector.memset(ones_sb, 1.0)
            for kt in range(NKT):
                nc.vector.tensor_copy(vsb[:, kt, :, 64], ones_sb)
            f32r = mybir.dt.float32r
            # lhsT of the rcp broadcast MM (f32r memset mis-compiles, so
            # memset f32 and copy)
            ones_f = keep.tile([1, 64], f32)
            nc.vector.memset(ones_f, 1.0)
            ones_r = keep.tile([1, 64], f32r)
            nc.vector.tensor_copy(ones_r, ones_f)

            # --- phase 1: projections ---
            with (
                tc.tile_pool(name="wpool", bufs=1) as wpool,
                tc.tile_pool(name="xpool", bufs=3) as xpool,
                tc.tile_pool(name="pp", bufs=2, space="PSUM") as pp,
            ):
                wq_t = wpool.tile([128, KC, DLOC], bf16, tag="wq")
                wk_t = wpool.tile([128, KC, DLOC], bf16, tag="wk")
                wv_t = wpool.tile([128, KC, DLOC], bf16, tag="wv")
                for w_t, w_d, w_q_ in ((wq_t, wq, nc.sync),
                                       (wk_t, wk, nc.gpsimd),
                                       (wv_t, wv, nc.scalar)):
                    w_q_.dma_start(out=w_t, in_=w_d[:, :, :])

                # round-robin input halves over the 3 DMA-capable queues so
                # the first tile lands fast and streams run in parallel
                dmaq = [nc.sync, nc.gpsimd, nc.scalar]
                qi = [0]

                def xfetch(x_d, nt):
                    xc = xpool.tile([128, KC, QT], bf16, tag="xc")
                    for h in range(2):
                        dmaq[qi[0] % 3].dma_start(
                            out=xc[:, h * (KC // 2):(h + 1) * (KC // 2), :],
                            in_=x_d[nt, :, h * (KC // 2):(h + 1) * (KC // 2),
                                    :])
                        qi[0] += 1
                    return xc

                for x_d, w_t, o_t, b_t in (
                    (xq, wq_t, qh, bq_t),
                    (xk, wk_t, kh, bk_t),
                ):
                    for nt in range(NQT):
                        xc = xfetch(x_d, nt)
                        for mt in range(2):
                            ps = pp.tile([128, QT], f32, tag="p")
                            for c in range(KC):
                                nc.tensor.matmul(
                                    ps,
                                    w_t[:, c, mt * 128:(mt + 1) * 128],
                                    xc[:, c, :],
                                    start=(c == 0), stop=(c == KC - 1))
                            nc.vector.tensor_scalar_add(
                                o_t[:, mt, nt * QT:(nt + 1) * QT],
                                ps, b_t[:, mt:mt + 1])

                # V projection, natural layout, per-head slices into vsb
                for nt in range(NQT):
                    xc = xfetch(xv, nt)
                    for stl in range(QT // 128):
                        st = nt * (QT // 128) + stl
                        psv = pp.tile([128, DLOC], f32, tag="pv")
                        for c in range(KC):
                            nc.tensor.matmul(
                                psv,
                                xc[:, c, stl * 128:(stl + 1) * 128],
                                wv_t[:, c, :],
                                start=(c == 0), stop=(c == KC - 1))
                        nc.vector.tensor_copy(
                            vsb[:, st, :, 0:64],
                            psv[:].rearrange("p (h d) -> p h d", h=4))

            # --- phase 2: attention, software-pipelined ---
            with (
                tc.tile_pool(name="att", bufs=2) as att,
                tc.tile_pool(name="avp", bufs=2) as avp,
                tc.tile_pool(name="psS", bufs=2, space="PSUM") as psS,
                tc.tile_pool(name="psA", bufs=1, space="PSUM") as psA,
                tc.tile_pool(name="psY", bufs=2, space="PSUM") as psY,
            ):
                def outproj(qt, mt, tail=False):
                    py = psY.tile([128, QT], f32, tag="py")
                    for c in range(2):
                        nc.tensor.matmul(
                            py,
                            wo_t[:, c, mt * 128:(mt + 1) * 128],
                            aoT[:, c, qt * QT:(qt + 1) * QT],
                            start=(c == 0), stop=(c == 1))
                    ysb = ypool.tile([128, QT], bf16, tag="y")
                    if tail and mt % 2 == 1:
                        # tail only: ScalarE is idle there, and Copy lives
                        # in every activation table set (no reload)
                        nc.scalar.copy(ysb, py)
                    else:
                        nc.vector.tensor_copy(ysb, py)
                    (nc.gpsimd if mt % 2 == 0 else nc.sync).dma_start(
                        out=yT[mt, qt], in_=ysb)

                def normalize(qt, hp, side, avs, rcr):
                    # rcp broadcast via a K=1 matmul (replaces the slow
                    # 64-descriptor broadcast DMA): bps[m, q] = rcp[q]
                    bps = psY.tile([128, QT], f32, tag="py")
                    nc.tensor.matmul(bps[0:64, :], ones_r, rcr,
                                     start=True, stop=True)
                    if side == 0:
                        nc.vector.tensor_mul(
                            aoT[0:64, hp, qt * QT:(qt + 1) * QT],
                            avs[0:64, :], bps[0:64, :])
                    else:
                        scr = small.tile([64, QT], bf16, tag="scr")
                        nc.vector.tensor_mul(scr, avs[0:64, :], bps[0:64, :])
                        nc.sync.dma_start(
                            out=aoT[64:128, hp, qt * QT:(qt + 1) * QT],
                            in_=scr)

                # one flat software-pipelined stream across all 8 blocks:
                # scores/exp for step f, attnV lagging LAG steps (crossing
                # block boundaries, so the exp stream never pauses), the
                # previous block's normalize at steps 5/7, out-proj of the
                # previous q tile at steps 8..11
                NB = NQT * 2
                pending = []
                state = {}

                for f in range(NB * NKT + LAG):
                    bi, s = divmod(f, NKT)
                    if bi < NB:
                        qt, hp = divmod(bi, 2)
                        if s == 0:
                            state[bi] = (
                                att.tile([128, NKT, 2, QT], bf16,
                                         tag="esb", name="esb"),
                                psA.tile([128, QT], f32, tag="av0",
                                         name="av0"),
                                psA.tile([128, QT], f32, tag="av1",
                                         name="av1"))
                        esb = state[bi][0]
                        kt = s
                        sc = psS.tile([128, 2 * QT], f32, tag="sc")
                        nc.tensor.matmul(
                            sc[:, 0:QT],
                            kh[0:64, hp, kt * 128:(kt + 1) * 128],
                            qh[0:64, hp, qt * QT:(qt + 1) * QT],
                            start=True, stop=True)
                        nc.tensor.matmul(
                            sc[:, QT:2 * QT],
                            kh[64:128, hp, kt * 128:(kt + 1) * 128],
                            qh[64:128, hp, qt * QT:(qt + 1) * QT],
                            start=True, stop=True)
                        nc.scalar.activation(
                            esb[:, kt, :, :], sc, AF.Exp, scale=0.125)
                        if s == 5 and pending:
                            pending.pop(0)()
                        if s == 7 and pending:
                            pending.pop(0)()
                        if qt >= 1 and 8 <= s < 12:
                            outproj(qt - 1, (s - 8) + 4 * hp)
                    f2 = f - LAG
                    if f2 >= 0:
                        bi2, kt2 = divmod(f2, NKT)
                        qt2, hp2 = divmod(bi2, 2)
                        esb2, av0, av1 = state[bi2]
                        nc.tensor.matmul(
                            av0[0:65, :], vsb[:, kt2, 2 * hp2, :],
                            esb2[:, kt2, 0, :],
                            start=(kt2 == 0), stop=(kt2 == NKT - 1),
                            skip_group_check=True)
                        nc.tensor.matmul(
                            av1[0:65, :], vsb[:, kt2, 2 * hp2 + 1, :],
                            esb2[:, kt2, 1, :],
                            start=(kt2 == 0), stop=(kt2 == NKT - 1),
                            skip_group_check=True)
                        if kt2 == NKT - 1:
                            # block epilogue: evacuate both av banks first
                            # (single DVE op each frees the bank), then the
                            # reciprocal chain; normalize is deferred
                            avss = []
                            for av in (av0, av1):
                                avs = avp.tile([128, QT], f32, tag="avs")
                                nc.vector.tensor_copy(avs, av)
                                avss.append(avs)
                            for side, avs in enumerate(avss):
                                # denominator to partition 0 (custom-DVE
                                # recip mislowers partition-offset inputs)
                                den = small.tile([1, QT], f32, tag="den")
                                nc.vector.tensor_copy(den, avs[64:65, :])
                                rcp = small.tile([1, QT], f32, tag="rcp")
                                nc.vector.reciprocal_approx_fast(rcp, den)
                                rcr = small.tile([1, QT], f32r, tag="rcr")
                                nc.vector.tensor_copy(rcr, rcp)
                                pending.append(
                                    lambda q_=qt2, h_=hp2, s_=side,
                                    a_=avs, r_=rcr:
                                    normalize(q_, h_, s_, a_, r_))
                            del state[bi2]

                # final flush: last block's normalize + last q tile out-proj
                for fn in pending:
                    fn()
                for mt in range(8):
                    outproj(NQT - 1, mt, tail=True)
    nc.compile()
    return nc


def _get_nc():
    if "nc" not in _CACHE:
        _CACHE["nc"] = _build()
    return _CACHE["nc"]


def kernel(q, k, v, w_q, b_q, w_k, b_k, w_v, b_v, w_o, b_o, _trace=False):
    from concourse.bass_utils import run_bass_kernel_spmd

    q = np.asarray(q, np.float32)
    k = np.asarray(k, np.float32)
    v = np.asarray(v, np.float32)
    w_q = np.asarray(w_q, np.float32)
    w_k = np.asarray(w_k, np.float32)
    w_v = np.asarray(w_v, np.float32)
    w_o = np.asarray(w_o, np.float32)
    b_q = np.asarray(b_q, np.float32)
    b_k = np.asarray(b_k, np.float32)
    b_v = np.asarray(b_v, np.float32)
    b_o = np.asarray(b_o, np.float32)

    nc = _get_nc()

    def tile_x(x):
        # [S, D] -> [NQT, 128, KC, QT]: A[nt, p, c, s] = x[nt*QT+s, c*128+p]
        t = x.T.reshape(KC, 128, NQT, QT)
        return np.ascontiguousarray(
            t.transpose(2, 1, 0, 3)).astype(BF16)

    def tile_w(w, lo, hi):
        # [D, dloc] -> [128, KC, dloc]
        t = w[lo:hi, :].T.reshape(KC, 128, DLOC)
        return np.ascontiguousarray(t.transpose(1, 0, 2)).astype(BF16)

    xqT = [tile_x(q[b]) for b in range(B)]
    xkT = [tile_x(k[b]) for b in range(B)]
    xvT = [tile_x(v[b]) for b in range(B)]

    in_maps = []
    for c in range(N_CORES):
        b, hg = c // 4, c % 4
        lo, hi = hg * DLOC, (hg + 1) * DLOC
        in_maps.append({
            "xq": xqT[b],
            "xk": xkT[b],
            "xv": xvT[b],
            "wq": tile_w(w_q, lo, hi),
            "wk": tile_w(w_k, lo, hi),
            "wv": tile_w(w_v, lo, hi),
            "wo": np.ascontiguousarray(
                w_o[:, lo:hi].T.reshape(2, 128, D).transpose(1, 0, 2)
            ).astype(BF16),
            "bq": np.ascontiguousarray(b_q[lo:hi].reshape(2, 128).T),
            "bk": np.ascontiguousarray(b_k[lo:hi].reshape(2, 128).T),
        })

    res = run_bass_kernel_spmd(
        nc, in_maps, core_ids=list(range(N_CORES)), trace=_trace)
    if _trace:
        _CACHE["last_result"] = res

    # b_v contributes exactly (w_o @ b_v) per output element (softmax rows
    # sum to 1); b_o adds directly.
    const_row = (b_o + w_o @ b_v).astype(np.float32)  # [D]
    out = np.empty((B, S, D), np.float32)
    for b in range(B):
        acc = res.results[4 * b]["yT"].astype(np.float32)
        for c in range(4 * b + 1, 4 * b + 4):
            acc += res.results[c]["yT"].astype(np.float32)
        # yT[mt, qt, p, s] = y_part[mt*128+p, qt*QT+s]
        y = acc.transpose(0, 2, 1, 3).reshape(D, S)
        out[b] = y.T + const_row
    return out
